# revision 1
# baseline (speedup 1.0000x reference)
"""GQA attention kernel for 8 trn2 NeuronCores — v2.2.

Sharding: core c handles batch b=c//2 and heads h0=(c%2)*8 .. h0+8.

Design (cost-model-driven; the Act-engine exp stream ~266us is the wall):
- Q/K projection in fp8e4 DoubleRow (x, Wq/Wk pair-interleaved over the
  contraction dim; W prescaled x16 to dodge fp8 subnormals, folded into the
  exp scale).
- RoPE on the f32 psum output in bf16 (per-head [evens|odds] 32-partition
  blocks), finishing with fused cross-partition adds that write the fp8 q/k
  tiles directly.  Per head-pair m a private [64, 2, T] fp8 tile: slot
  offsets {0,32}, hd-halves as the DoubleRow pair dim — chains never touch
  other phases' tiles (no false deps).
- Scores: fp8 DoubleRow, contraction 32x2, out [128 keys, 512 q] per
  (head, kt) — exactly one psum bank per matmul group.
- exp on Act only, 256 x [128,1024] instrs, scale folded.
- AV bf16 O^T with ones-column denominators; out-projection bf16, y in bf16.
- Flat (qc, j, kt) stream at the ~1.04us/step exp cadence; AV lags scores
  by 4 steps; RoPE chains and split out-proj chunks ride as fillers.

PSUM banks: sc 2x[128,1024]=4, otA+otB=2, acc x2=2 (exactly 8).
"""
import sys
sys.path.insert(0, "/opt/trn_rl_repo")
from collections import deque
import numpy as np
import ml_dtypes
import concourse.bacc as bacc
import concourse.mybir as mybir
import concourse.tile as tile
from concourse.bass_utils import run_bass_kernel_spmd

B, T, D = 4, 2048, 1024
HD = 64
P = 128
QC = 512             # query chunk
NQC = T // QC        # 4
KT = T // P          # 16 key tiles
NH = 8               # heads per core
WPRE = 16.0
SCALE = 1.0 / (float(np.sqrt(512.0)) * WPRE * WPRE)

f32 = mybir.dt.float32
bf16 = mybir.dt.bfloat16
fp8 = mybir.dt.float8e4
EXP = mybir.ActivationFunctionType.Exp
DR = mybir.MatmulPerfMode.DoubleRow

_PERM = np.concatenate([np.arange(0, HD, 2), np.arange(1, HD, 2)])


def _build_nc():
    nc = bacc.Bacc("TRN2", target_bir_lowering=False)
    xp8 = nc.dram_tensor("xp8", [P, 4 * 2 * T], fp8, kind="ExternalInput")
    xbig = nc.dram_tensor("xbig", [P, 8 * T], bf16, kind="ExternalInput")
    wq8 = nc.dram_tensor("wq8", [P, 4 * 4 * 2 * P], fp8, kind="ExternalInput")
    wk8 = nc.dram_tensor("wk8", [P, 4 * 4 * 2 * P], fp8, kind="ExternalInput")
    wv = nc.dram_tensor("wv", [P, 8 * 512], bf16, kind="ExternalInput")
    wp = nc.dram_tensor("wp", [512, D], bf16, kind="ExternalInput")
    cosT = nc.dram_tensor("cosT", [P, T], bf16, kind="ExternalInput")
    sinT = nc.dram_tensor("sinT", [P, T], bf16, kind="ExternalInput")
    y = nc.dram_tensor("y", [T, D], bf16, kind="ExternalOutput")

    with tile.TileContext(nc) as tc:
        with (
            tc.tile_pool(name="persist", bufs=1) as pp,
            tc.tile_pool(name="a2p", bufs=20) as a2p,
            tc.tile_pool(name="rope", bufs=2) as rp,
            tc.tile_pool(name="vst", bufs=2) as vstp,
            tc.tile_pool(name="nrm", bufs=2) as np_,
            tc.tile_pool(name="ontp", bufs=2) as ontp,
            tc.tile_pool(name="ysp", bufs=8) as ysp,
            tc.tile_pool(name="scp", bufs=2, space="PSUM") as scp,
            tc.tile_pool(name="otp", bufs=1, space="PSUM") as otp,
            tc.tile_pool(name="accp", bufs=2, space="PSUM") as accp,
        ):
            # ---- persistent tiles + input DMA (carefully ordered) ----
            wk8t = pp.tile([P, 4, 4, 2, P], fp8, tag="wk8t", name="wk8t")
            nc.sync.dma_start(out=wk8t[:, 0, :, :, :], in_=wk8[:, 0:1024])
            xp8t = []
            for kp in range(4):
                t = pp.tile([P, 2, T], fp8, tag=f"xp8_{kp}", name=f"xp8_{kp}")
                nc.sync.dma_start(out=t[:], in_=xp8[:, kp * 2 * T:(kp + 1) * 2 * T])
                xp8t.append(t)
            wq8t = pp.tile([P, 4, 4, 2, P], fp8, tag="wq8t", name="wq8t")
            nc.sync.dma_start(out=wq8t[:, 0, :, :, :], in_=wq8[:, 0:1024])
            nc.sync.dma_start(out=wk8t[:, 1:4, :, :, :], in_=wk8[:, 1024:4096])
            nc.sync.dma_start(out=wq8t[:, 1:4, :, :, :], in_=wq8[:, 1024:4096])
            tcos = pp.tile([P, T], bf16, tag="tcos", name="tcos")
            nc.sync.dma_start(out=tcos[:], in_=cosT[:])
            tsin = pp.tile([P, T], bf16, tag="tsin", name="tsin")
            nc.sync.dma_start(out=tsin[:], in_=sinT[:])
            wvt = pp.tile([P, 8, 512], bf16, tag="wvt", name="wvt")
            nc.sync.dma_start(out=wvt[:], in_=wv[:])
            # x for V-proj: one [128, 8, 2048] tile, DMA'd in 4 column chunks
            xbt = pp.tile([P, 8, T], bf16, tag="xbt", name="xbt")
            xbig3 = xbig.rearrange("p (k t) -> p k t", k=8)
            for c in range(4):
                nc.sync.dma_start(out=xbt[:, :, c * QC:(c + 1) * QC],
                                  in_=xbig3[:, :, c * QC:(c + 1) * QC])
            wps = []
            for j in range(4):
                t = pp.tile([P, D], bf16, tag=f"wp{j}", name=f"wp{j}")
                nc.sync.dma_start(out=t[:], in_=wp[j * P:(j + 1) * P, :])
                wps.append(t)

            # per head-pair private fp8 q/k tiles: [64, 2, T], slots {0,32}
            qt8 = [pp.tile([64, 2, T], fp8, tag=f"qt8_{m}", name=f"qt8_{m}")
                   for m in range(4)]
            kt8 = [pp.tile([64, 2, T], fp8, tag=f"kt8_{m}", name=f"kt8_{m}")
                   for m in range(4)]
            va = [pp.tile([P, 520], bf16, tag=f"va{kt}", name=f"va{kt}")
                  for kt in range(KT)]

            ont_of = {}

            # ---- emission helpers ----
            def qk_rope(dst, w8t, m, qc):
                """project m-tile (heads 2m,2m+1), tokens qc*512.., + RoPE,
                write fp8 slots of dst[m] via fused cross-partition adds."""
                qs = slice(qc * QC, (qc + 1) * QC)
                ps = accp.tile([P, QC], f32, tag="acc", name="qkps")
                for kp in range(4):
                    nc.tensor.matmul(ps[:], w8t[:, m, kp, :, :],
                                     xp8t[kp][:, :, qs],
                                     start=(kp == 0), stop=(kp == 3),
                                     perf_mode=DR)
                qsb = rp.tile([P, QC], bf16, tag="qsb", name="qsb")
                nc.vector.tensor_copy(qsb[:], ps[:])
                rot = rp.tile([P, QC], bf16, tag="rot", name="rot")
                for blk in range(4):
                    s = (blk ^ 1) * 32
                    eng = nc.vector if blk < 2 else nc.gpsimd
                    eng.tensor_copy(rot[blk * 32:(blk + 1) * 32, :],
                                    qsb[s:s + 32, :])
                t0 = rp.tile([P, QC], bf16, tag="t0", name="t0")
                nc.vector.tensor_mul(t0[:], qsb[:], tcos[:, qs])
                t1 = rp.tile([P, QC], bf16, tag="t1", name="t1")
                nc.vector.tensor_mul(t1[:], rot[:], tsin[:, qs])
                for blk in range(4):
                    hp = blk // 2          # head-in-pair
                    pr = blk % 2           # hd-half = pair slot
                    bs = slice(32 * blk, 32 * blk + 32)
                    eng = nc.gpsimd if blk == 3 else nc.vector
                    eng.tensor_add(dst[m][32 * hp:32 * hp + 32, pr, qs],
                                   t0[bs, :], t1[bs, :])

            def scores(j, qc, kt):
                sc = scp.tile([P, 2 * QC], f32, tag="sc", name="sc")
                qs = slice(qc * QC, (qc + 1) * QC)
                ks = slice(kt * P, (kt + 1) * P)
                for hp in range(2):
                    sl = slice(32 * hp, 32 * hp + 32)
                    nc.tensor.matmul(sc[:, hp * QC:(hp + 1) * QC],
                                     kt8[j][sl, :, ks], qt8[j][sl, :, qs],
                                     start=True, stop=True, perf_mode=DR)
                a2t = a2p.tile([P, 2 * QC], bf16, tag="a2", name="a2")
                nc.scalar.activation(a2t[:], sc[:], EXP, scale=SCALE)
                return a2t

            def av(j, kt, a2t, otA, otB):
                nc.tensor.matmul(otA[0:65, :],
                                 va[kt][:, (2 * j) * 65:(2 * j) * 65 + 65],
                                 a2t[:, 0:QC],
                                 start=(kt == 0), stop=(kt == KT - 1))
                nc.tensor.matmul(otB[0:65, :],
                                 va[kt][:, (2 * j + 1) * 65:(2 * j + 1) * 65 + 65],
                                 a2t[:, QC:2 * QC],
                                 start=(kt == 0), stop=(kt == KT - 1))

            def vproj(mt, wide=False):
                # in j0 the AV accumulators are idle (drains deferred): cycle
                # psum across 4 banks so DVE-evac latency never stalls PE
                if wide and mt % 2 == 1:
                    tag = "otA" if mt % 4 == 1 else "otB"
                    ps = otp.tile([P, QC], f32, tag=tag, name="vps")
                else:
                    ps = accp.tile([P, QC], f32, tag="acc", name="vps")
                for k in range(8):
                    nc.tensor.matmul(ps[:], xbt[:, k, mt * P:(mt + 1) * P],
                                     wvt[:, k, :],
                                     start=(k == 0), stop=(k == 7))
                vs = vstp.tile([P, QC], bf16, tag="vs", name="vs")
                nc.vector.tensor_copy(vs[:], ps[:])
                # 65-stride re-layout rides the idle DMA engines
                nc.sync.dma_start(
                    out=va[mt][:].rearrange("p (h d) -> p h d", h=8)[:, :, 0:64],
                    in_=vs[:].rearrange("p (h d) -> p h d", h=8))

            def normalize(j, qc, otA, otB):
                ont_t = ontp.tile([P, QC], bf16, tag=f"ont{j}", name=f"ont{j}")
                ont_of[(qc, j)] = ont_t
                for ot, off in ((otA, 0), (otB, 64)):
                    r = np_.tile([1, QC], f32, tag="r", name="r")
                    nc.vector.reciprocal(r[:], ot[64:65, :])
                    rb = np_.tile([64, QC], f32, tag="rb", name="rb")
                    nc.gpsimd.partition_broadcast(rb[:], r[:])
                    nc.vector.tensor_mul(ont_t[off:off + 64, :],
                                         ot[0:64, :], rb[:])

            acc_of = {}

            def outproj_half(qc, mtl, nt, half):
                if half == 0:
                    ps = accp.tile([P, QC], f32, tag="acc", name="yps")
                    acc_of[(mtl, nt)] = ps
                else:
                    ps = acc_of.pop((mtl, nt))
                for jj in (0, 1) if half == 0 else (2, 3):
                    nc.tensor.matmul(ps[:],
                                     ont_of[(qc, jj)][:, mtl * P:(mtl + 1) * P],
                                     wps[jj][:, nt * QC:(nt + 1) * QC],
                                     start=(jj == 0), stop=(jj == 3))
                if half == 1:
                    ys = ysp.tile([P, QC], bf16, tag="ys", name="ys")
                    nc.vector.tensor_copy(ys[:], ps[:])
                    mt = qc * 4 + mtl
                    nc.sync.dma_start(
                        out=y[mt * P:(mt + 1) * P, nt * QC:(nt + 1) * QC],
                        in_=ys[:])

            # ---- prefix: K(m0) + Q0(m0) chains only; V rides inside j0.
            # qck0 + Q first so the first scores unblock earliest.
            qk_rope(kt8, wk8t, 0, 0)
            qk_rope(qt8, wq8t, 0, 0)
            for qck in range(1, NQC):
                qk_rope(kt8, wk8t, 0, qck)
            # ones columns for va, off the chains' critical path
            for kt in range(KT):
                nc.gpsimd.memset(va[kt][:], 1.0)

            # ---- steady state: flat (qc, j, kt) stream, AV lag 4 ----
            pend = deque()
            cur_ot = [None, None]

            def drain_one():
                j, qc, kt, a2t = pend.popleft()
                if kt == 0:
                    cur_ot[0] = otp.tile([P, QC], f32, tag="otA", name="otA")
                    cur_ot[1] = otp.tile([P, QC], f32, tag="otB", name="otB")
                av(j, kt, a2t, cur_ot[0], cur_ot[1])
                if kt == KT - 1:
                    normalize(j, qc, cur_ot[0], cur_ot[1])

            def filler(qc, j, kt):
                if qc == 0 and j == 0:
                    # V-proj kt 0..12 rides in j0 (no AV drains here);
                    # K(m1)/Q0(m1) chains front-loaded so j1 scores can
                    # follow j0 seamlessly
                    if kt <= 12:
                        vproj(kt, wide=True)
                    if kt in (1, 3, 5, 7):
                        qk_rope(kt8, wk8t, 1, (kt - 1) // 2)
                    elif kt == 9:
                        qk_rope(qt8, wq8t, 1, 0)
                if qc == 0 and j == 1 and kt in (1, 3, 5):
                    vproj(13 + (kt - 1) // 2)
                if qc == 0 and j in (1, 2):
                    # K ropes for m=j+1
                    if kt in (0, 4, 8, 12):
                        qk_rope(kt8, wk8t, j + 1, kt // 4)
                    elif kt == 2:
                        qk_rope(qt8, wq8t, j + 1, 0)
                elif qc >= 1 and j in (1, 2):
                    if kt % 2 == 1:
                        idx = (j - 1) * 8 + (kt - 1) // 2   # 0..15
                        mtl = idx // 4
                        nt = (idx // 2) % 2
                        outproj_half(qc - 1, mtl, nt, idx % 2)
                if qc == 0:
                    if j == 3 and kt in (1, 5, 9, 13):
                        qk_rope(qt8, wq8t, (kt - 1) // 4, qc + 1)
                elif qc < NQC - 1:
                    if j == 2 and kt in (6, 14):
                        qk_rope(qt8, wq8t, (kt - 6) // 8, qc + 1)
                    if j == 3 and kt in (1, 9):
                        qk_rope(qt8, wq8t, (kt - 1) // 8 + 2, qc + 1)

            for qc in range(NQC):
                for j in range(4):
                    for kt in range(KT):
                        # drains + fillers first: their deps are satisfied, so
                        # the in-order PE works instead of blocking on the
                        # next scores psum bank
                        if not (qc == 0 and j == 0):
                            if qc == 0 and j == 1:
                                n = 1
                            elif qc == NQC - 1 and j == 3:
                                # run the AV tail tight so the post-stream
                                # drain backlog is minimal
                                n = min(2, max(0, len(pend) - 2))
                            else:
                                n = 2 if len(pend) > 5 else (1 if len(pend) > 4 else 0)
                            for _ in range(n):
                                drain_one()
                        filler(qc, j, kt)
                        a2t = scores(j, qc, kt)
                        pend.append((j, qc, kt, a2t))
            while pend:
                drain_one()
            # tail out-proj of the last qc: reuse the idle scores banks to
            # pipeline 4+ chunks at once
            qc3 = NQC - 1
            tail_ps = []
            for i in range(2):
                sct = scp.tile([P, 2 * QC], f32, tag="sc", name=f"tsc{i}")
                tail_ps.append(sct[:, 0:QC])
                tail_ps.append(sct[:, QC:2 * QC])
            tail_ps.append(accp.tile([P, QC], f32, tag="acc", name="tacc0")[:])
            tail_ps.append(accp.tile([P, QC], f32, tag="acc", name="tacc1")[:])
            tail_ps.append(otp.tile([P, QC], f32, tag="otA", name="totA")[:])
            tail_ps.append(otp.tile([P, QC], f32, tag="otB", name="totB")[:])
            chunks = [(m, n) for m in range(4) for n in range(2)]
            for jj in range(4):
                for ci, (mtl, nt) in enumerate(chunks):
                    nc.tensor.matmul(tail_ps[ci],
                                     ont_of[(qc3, jj)][:, mtl * P:(mtl + 1) * P],
                                     wps[jj][:, nt * QC:(nt + 1) * QC],
                                     start=(jj == 0), stop=(jj == 3))
            for ci, (mtl, nt) in enumerate(chunks):
                ys = ysp.tile([P, QC], bf16, tag="ys", name="ys")
                if ci % 2 == 0:
                    nc.vector.tensor_copy(ys[:], tail_ps[ci])
                else:
                    nc.scalar.copy(ys[:], tail_ps[ci])
                mt = qc3 * 4 + mtl
                nc.sync.dma_start(
                    out=y[mt * P:(mt + 1) * P, nt * QC:(nt + 1) * QC],
                    in_=ys[:])

    nc.compile()
    return nc


_NC_CACHE = None


def _rope_tables():
    """cos/sin tables for the [evens(32)|odds(32)] per-head psum layout.
    Row p uses theta_(p%32); sin sign is - for the evens half."""
    thetas = 1000.0 ** (-2.0 * np.arange(1, 33, dtype=np.float64) / 64.0)
    pos = np.arange(1, T + 1, dtype=np.float64)
    args = pos[None, :] * thetas[:, None]          # [32, T]
    cos32 = np.cos(args)
    sin32 = np.sin(args)
    bf = ml_dtypes.bfloat16
    cos128 = np.tile(cos32, (4, 1)).astype(bf)
    sin128 = np.concatenate([-sin32, sin32, -sin32, sin32], axis=0).astype(bf)
    return np.ascontiguousarray(cos128), np.ascontiguousarray(sin128)


def kernel(x, W_attn, b_attn, W_proj, b_proj):
    global _NC_CACHE
    x = np.asarray(x, dtype=np.float32)
    W_attn = np.asarray(W_attn, dtype=np.float32)
    W_proj = np.asarray(W_proj, dtype=np.float32)
    b_proj = np.asarray(b_proj, dtype=np.float32)
    bf = ml_dtypes.bfloat16
    f8 = ml_dtypes.float8_e4m3
    cos128, sin128 = _rope_tables()

    in_maps = []
    for c in range(8):
        b = c // 2
        h0 = (c % 2) * 8
        qcols = np.concatenate([h * HD + _PERM for h in range(h0, h0 + 8)])
        vcols = np.arange(h0 * HD, (h0 + 8) * HD)

        xTb = np.ascontiguousarray(x[b].T)                       # [1024, 2048]
        xp8 = np.ascontiguousarray(
            xTb.reshape(4, 2, 128, T).transpose(2, 0, 1, 3).reshape(P, 4 * 2 * T)
        ).astype(f8)
        xbig = np.ascontiguousarray(
            xTb.reshape(8, 128, T).transpose(1, 0, 2).reshape(P, 8 * T)
        ).astype(bf)

        def packw(Wsub):
            Ws = np.ascontiguousarray(Wsub[:, qcols]) * WPRE     # [1024, 512]
            return np.ascontiguousarray(
                Ws.reshape(4, 2, 128, 4, 128).transpose(2, 3, 0, 1, 4)
                .reshape(P, 4 * 4 * 2 * P)).astype(f8)

        Wvs = np.ascontiguousarray(W_attn[:, 2048:3072][:, vcols])  # [1024, 512]
        wvbig = np.ascontiguousarray(
            Wvs.reshape(8, 128, 512).transpose(1, 0, 2).reshape(P, 8 * 512)
        ).astype(bf)

        in_maps.append({
            "xp8": xp8,
            "xbig": xbig,
            "wq8": packw(W_attn[:, 0:1024]),
            "wk8": packw(W_attn[:, 1024:2048]),
            "wv": wvbig,
            "wp": np.ascontiguousarray(W_proj[vcols, :]).astype(bf),
            "cosT": cos128,
            "sinT": sin128,
        })

    if _NC_CACHE is None:
        _NC_CACHE = _build_nc()
    import os
    trace = bool(os.environ.get("KERNEL_TRACE"))
    kw = {}
    if trace:
        tdir = os.environ.get("KERNEL_TRACE_DIR") or None
        kw = dict(trace=True, tmpdir=tdir)
    res = run_bass_kernel_spmd(_NC_CACHE, in_maps, list(range(8)), **kw)
    if trace and res.exec_time_ns is not None:
        print(f"HW exec time: {res.exec_time_ns} ns")
    out = np.empty((B, T, D), dtype=np.float32)
    for b in range(B):
        out[b] = (res.results[2 * b]["y"].astype(np.float32)
                  + res.results[2 * b + 1]["y"].astype(np.float32)
                  + b_proj[None, :])
    return out



# revision 46
# speedup vs baseline: 1.0427x; 1.0427x over previous
"""GQA attention kernel for 8 trn2 NeuronCores — v3.0 (schedule compaction).

Sharding: core c handles batch b=c//2 and heads h0=(c%2)*8 .. h0+8.

Design (cost-model-driven; the Act-engine exp stream ~266us is the wall):
- Q/K projection in fp8e4 DoubleRow (x, Wq/Wk pair-interleaved over the
  contraction dim; W prescaled x16 to dodge fp8 subnormals, folded into the
  exp scale).
- RoPE on the f32 psum output in bf16 (per-head [evens|odds] 32-partition
  blocks), finishing with fused cross-partition adds that write the fp8 q/k
  tiles directly.
- Scores: fp8 DoubleRow, contraction 32x2, out [128 keys, 1024 q] per
  (head-pair, kt) — one psum bank pair per matmul group.
- exp on Act only, 256 x [128,1024] instrs, scale folded.
- AV bf16 O^T with ones-column denominators; out-projection bf16, y in bf16.

v3.0 changes vs v2.2 (all schedule, no numerics):
- DMA stream reordered critical-first and chunked by qc (xp8 packed
  [P,kp,qc,2,512] host-side) so the first exp fires at ~10us not ~20us.
- First K/Q chains get a 2+2 DVE/Pool add split (latency); several qc0
  chains evac their proj psum on gpsimd (DVE relief).
- vproj gated on xbt chunk arrival, spread j0..j2 with explicit qc0 drain
  cadence so PE never floods.
- Tail: qc3 out-proj jj0+jj1 ride j3 into SBUF f32 temps; jj2+jj3 finish
  post-stream with a fused add-evac. Act's stream ends at the last exp.

PSUM banks: sc 2x[128,1024]=4, otA+otB=2, acc x2=2 (exactly 8).
"""
import sys
sys.path.insert(0, "/opt/trn_rl_repo")
from collections import deque
import numpy as np
import ml_dtypes
import concourse.bacc as bacc
import concourse.mybir as mybir
import concourse.tile as tile
from concourse.bass_utils import run_bass_kernel_spmd

B, T, D = 4, 2048, 1024
HD = 64
P = 128
QC = 512             # query chunk
NQC = T // QC        # 4
KT = T // P          # 16 key tiles
NH = 8               # heads per core
WPRE = 16.0
SCALE = 1.0 / (float(np.sqrt(512.0)) * WPRE * WPRE)

f32 = mybir.dt.float32
bf16 = mybir.dt.bfloat16
fp8 = mybir.dt.float8e4
EXP = mybir.ActivationFunctionType.Exp
DR = mybir.MatmulPerfMode.DoubleRow

_PERM = np.concatenate([np.arange(0, HD, 2), np.arange(1, HD, 2)])


def _build_nc():
    nc = bacc.Bacc("TRN2", target_bir_lowering=False)
    # xp8 host layout: [P, kp, qc, slot, 512]
    xp8 = nc.dram_tensor("xp8", [P, 4 * 4 * 2 * QC], fp8, kind="ExternalInput")
    xbig = nc.dram_tensor("xbig", [P, 8 * T], bf16, kind="ExternalInput")
    wq8 = nc.dram_tensor("wq8", [P, 4 * 4 * 2 * P], fp8, kind="ExternalInput")
    wk8 = nc.dram_tensor("wk8", [P, 4 * 4 * 2 * P], fp8, kind="ExternalInput")
    wv = nc.dram_tensor("wv", [P, 8 * 512], bf16, kind="ExternalInput")
    wp = nc.dram_tensor("wp", [512, D], bf16, kind="ExternalInput")
    # cs packs [cos|sin]: [P, 2, T]
    csT = nc.dram_tensor("csT", [P, 2 * T], bf16, kind="ExternalInput")
    ident = nc.dram_tensor("ident", [P, P], bf16, kind="ExternalInput")
    y = nc.dram_tensor("y", [T, D], bf16, kind="ExternalOutput")

    xp8r = xp8.rearrange("p (k q s t) -> p k q s t", k=4, q=4, s=2)
    xbig3 = xbig.rearrange("p (k t) -> p k t", k=8)
    csr = csT.rearrange("p (c t) -> p c t", c=2)
    wpr = wp.rearrange("(j p) d -> p j d", j=4)

    with tile.TileContext(nc) as tc:
        with (
            tc.tile_pool(name="persist", bufs=1) as pp,
            tc.tile_pool(name="a2p", bufs=24) as a2p,
            tc.tile_pool(name="rope", bufs=2) as rp,
            tc.tile_pool(name="vst", bufs=2) as vstp,
            tc.tile_pool(name="nrm", bufs=2) as np_,
            tc.tile_pool(name="ontp", bufs=2) as ontp,
            tc.tile_pool(name="ysp", bufs=5) as ysp,
            tc.tile_pool(name="scp", bufs=2, space="PSUM") as scp,
            tc.tile_pool(name="otp", bufs=1, space="PSUM") as otp,
            tc.tile_pool(name="accp", bufs=2, space="PSUM") as accp,
        ):
            # ---- persistent tiles; DMA issue order is critical-first ----
            wk8t = pp.tile([P, 4, 4, 2, P], fp8, tag="wk8t", name="wk8t")
            wq8t = pp.tile([P, 4, 4, 2, P], fp8, tag="wq8t", name="wq8t")
            xp8a = pp.tile([P, 4, 4, 2, QC], fp8, tag="xp8a", name="xp8a")
            xp8t = [xp8a[:, kp] for kp in range(4)]
            cst = pp.tile([P, 2, T], bf16, tag="cst", name="cst")
            tcos = cst[:, 0]
            tsin = cst[:, 1]
            wvt = pp.tile([P, 8, 512], bf16, tag="wvt", name="wvt")
            xbt = pp.tile([P, 8, T], bf16, tag="xbt", name="xbt")
            wpt = pp.tile([P, 4, D], bf16, tag="wpt", name="wpt")
            wps = [wpt[:, j] for j in range(4)]

            def dma_x(qc):
                # one DMA covers all 4 kp chunks of this qc
                nc.sync.dma_start(out=xp8a[:, :, qc], in_=xp8r[:, :, qc])

            def dma_cs(qc):
                qs = slice(qc * QC, (qc + 1) * QC)
                nc.sync.dma_start(out=cst[:, :, qs], in_=csr[:, :, qs])

            def dma_xbt(c):
                nc.sync.dma_start(out=xbt[:, :, c * QC:(c + 1) * QC],
                                  in_=xbig3[:, :, c * QC:(c + 1) * QC])

            # critical chain inputs first, then interleave V/x with later qcs
            nc.sync.dma_start(out=wk8t[:], in_=wk8[:])
            dma_x(0)
            dma_cs(0)
            nc.sync.dma_start(out=wq8t[:], in_=wq8[:])
            dma_x(1)
            dma_cs(1)
            nc.sync.dma_start(out=wvt[:], in_=wv[:])
            dma_xbt(0)
            dma_x(2)
            dma_cs(2)
            dma_xbt(1)
            dma_x(3)
            dma_cs(3)
            dma_xbt(2)
            dma_xbt(3)
            nc.sync.dma_start(out=wpt[:], in_=wpr[:])
            idt = pp.tile([P, P], bf16, tag="idt", name="idt")
            nc.sync.dma_start(out=idt[:], in_=ident[:])

            # per head-pair private fp8 q/k tiles: [64, 2, T], slots {0,32}
            qt8 = [pp.tile([64, 2, T], fp8, tag=f"qt8_{m}", name=f"qt8_{m}")
                   for m in range(4)]
            kt8 = [pp.tile([64, 2, T], fp8, tag=f"kt8_{m}", name=f"kt8_{m}")
                   for m in range(4)]
            va = [pp.tile([P, 520], bf16, tag=f"va{kt}", name=f"va{kt}")
                  for kt in range(KT)]
            # ones columns for va: emitted early so they pack into the idle
            # prologue; split across DVE and gpsimd
            for kt in range(KT):
                (nc.vector if kt % 2 == 0 else nc.gpsimd).memset(
                    va[kt][:], 1.0)

            ont_of = {}

            # ---- emission helpers ----
            def qk_proj(w8t, m, qc):
                """Q/K projection matmuls for one m-tile/qc chunk -> psum."""
                ps = accp.tile([P, QC], f32, tag="acc", name="qkps")
                for kp in range(4):
                    nc.tensor.matmul(ps[:], w8t[:, m, kp, :, :],
                                     xp8t[kp][:, qc, :, :],
                                     start=(kp == 0), stop=(kp == 3),
                                     perf_mode=DR)
                return ps

            def rope_chain(dst, ps, m, qc, c0=0, c1=QC,
                           pool_adds=1):
                """RoPE on psum cols [c0:c1) of the m-tile, writing fp8 slots
                of dst[m]. Rotation block copies always ride DVE (4x mode);
                `pool_adds` of the 4 adds go to gpsimd (psum ops are
                DVE-only: GPSIMD cannot access PSUM)."""
                w = c1 - c0
                qs = slice(qc * QC + c0, qc * QC + c1)
                qsb = rp.tile([P, w], bf16, tag="qsb", name="qsb")
                nc.vector.tensor_copy(qsb[:], ps[:, c0:c1])
                rot = rp.tile([P, w], bf16, tag="rot", name="rot")
                for blk in range(4):
                    s = (blk ^ 1) * 32
                    nc.vector.tensor_copy(rot[blk * 32:(blk + 1) * 32, :],
                                          qsb[s:s + 32, :])
                t0 = rp.tile([P, w], bf16, tag="t0", name="t0")
                nc.vector.tensor_mul(t0[:], qsb[:], tcos[:, qs])
                t1 = rp.tile([P, w], bf16, tag="t1", name="t1")
                nc.vector.tensor_mul(t1[:], rot[:], tsin[:, qs])
                for blk in range(4):
                    hp = blk // 2          # head-in-pair
                    pr = blk % 2           # hd-half = pair slot
                    bs = slice(32 * blk, 32 * blk + 32)
                    eng = nc.gpsimd if blk >= 4 - pool_adds else nc.vector
                    eng.tensor_add(dst[m][32 * hp:32 * hp + 32, pr, qs],
                                   t0[bs, :], t1[bs, :])

            def qk_rope(dst, w8t, m, qc, pool_adds=1):
                ps = qk_proj(w8t, m, qc)
                rope_chain(dst, ps, m, qc, pool_adds=pool_adds)

            def scores(j, qc, kt):
                sc = scp.tile([P, 2 * QC], f32, tag="sc", name="sc")
                qs = slice(qc * QC, (qc + 1) * QC)
                ks = slice(kt * P, (kt + 1) * P)
                for hp in range(2):
                    sl = slice(32 * hp, 32 * hp + 32)
                    nc.tensor.matmul(sc[:, hp * QC:(hp + 1) * QC],
                                     kt8[j][sl, :, ks], qt8[j][sl, :, qs],
                                     start=True, stop=True, perf_mode=DR)
                a2t = a2p.tile([P, 2 * QC], bf16, tag="a2", name="a2")
                nc.scalar.activation(a2t[:], sc[:], EXP, scale=SCALE)
                return a2t

            def av(j, kt, a2t, otA, otB):
                # stationary = a2 q-block, moving = va head-slice: out free
                # is 65, so the cost-model charge halves vs the O^T form.
                # out: [128 q, 4 qsub, 65] per head, accumulated over kt.
                for h in (0, 1):
                    ot = (otA, otB)[h]
                    vs65 = va[kt][:, (2 * j + h) * 65:(2 * j + h) * 65 + 65]
                    for c in range(4):
                        blk = (h * 4 + c) * P
                        # start=True zeroes the whole bank: only the bank's
                        # first matmul may carry it
                        nc.tensor.matmul(ot[:, c, :],
                                         a2t[:, blk:blk + P], vs65,
                                         start=(kt == 0 and c == 0),
                                         stop=(kt == KT - 1),
                                         skip_group_check=True)

            def vproj(mt, wide=False):
                # when AV accumulators are idle (drains deferred): cycle
                # psum across 4 banks so DVE-evac latency never stalls PE
                if wide and mt % 2 == 1:
                    tag = "otA" if mt % 4 == 1 else "otB"
                    ps = otp.tile([P, QC], f32, tag=tag, name="vps")
                else:
                    ps = accp.tile([P, QC], f32, tag="acc", name="vps")
                for k in range(8):
                    nc.tensor.matmul(ps[:], xbt[:, k, mt * P:(mt + 1) * P],
                                     wvt[:, k, :],
                                     start=(k == 0), stop=(k == 7))
                vs = vstp.tile([P, QC], bf16, tag="vs", name="vs")
                nc.vector.tensor_copy(vs[:], ps[:])
                # 65-stride re-layout rides the idle DMA engines
                nc.sync.dma_start(
                    out=va[mt][:].rearrange("p (h d) -> p h d", h=8)[:, :, 0:64],
                    in_=vs[:].rearrange("p (h d) -> p h d", h=8))

            MUL = mybir.AluOpType.mult

            def normalize(j, qc, otA, otB):
                """evac1: per-row reciprocal of the denominator column and a
                per-partition-scalar multiply into the [q, h, c, hd] O tile,
                then PE-transpose back to O^T and evac2 into ont."""
                osb = ontp.tile([P, 2, 4, HD], bf16, tag="osb", name="osb")
                for h, ot in enumerate((otA, otB)):
                    r = np_.tile([P, 4], f32, tag="r", name="r")
                    nc.vector.reciprocal(r[:], ot[:, :, 64])
                    for c in range(4):
                        nc.vector.tensor_scalar(osb[:, h, c, :],
                                                ot[:, c, 0:HD],
                                                r[:, c:c + 1], None, MUL)
                ont_t = ontp.tile([P, QC], bf16, tag=f"ont{j}", name=f"ont{j}")
                ont_of[(qc, j)] = ont_t
                for h in (0, 1):
                    tag = ("otA", "otB")[h]
                    psT = otp.tile([HD, 4, P], bf16, tag=tag, name="psT")
                    for c in range(4):
                        nc.tensor.transpose(psT[:, c, :], osb[:, h, c, :],
                                            idt[:])
                    nc.vector.tensor_copy(
                        ont_t[h * HD:(h + 1) * HD, :],
                        psT[:].rearrange("p c q -> p (c q)"))

            acc_of = {}

            def outproj_half(qc, mtl, nt, half):
                if half == 0:
                    ps = accp.tile([P, QC], f32, tag="acc", name="yps")
                    acc_of[(mtl, nt)] = ps
                else:
                    ps = acc_of.pop((mtl, nt))
                for jj in (0, 1) if half == 0 else (2, 3):
                    nc.tensor.matmul(ps[:],
                                     ont_of[(qc, jj)][:, mtl * P:(mtl + 1) * P],
                                     wps[jj][:, nt * QC:(nt + 1) * QC],
                                     start=(jj == 0), stop=(jj == 3))
                if half == 1:
                    ys = ysp.tile([P, QC], bf16, tag="ys", name="ys")
                    nc.vector.tensor_copy(ys[:], ps[:])
                    mt = qc * 4 + mtl
                    nc.sync.dma_start(
                        out=y[mt * P:(mt + 1) * P, nt * QC:(nt + 1) * QC],
                        in_=ys[:])

            # ---- qc3 out-proj tail: jj0+jj1 ride j2/j3 into bf16 SBUF temps
            # (xbt is dead after vproj); jj2+jj3 finish post-stream on the
            # freed score banks with a fused add-evac.
            tail_sb = {}

            def tail_front(ci):
                qc3 = NQC - 1
                mtl, nt = ci // 2, ci % 2
                ps = accp.tile([P, QC], f32, tag="acc", name="tps")
                for jj in (0, 1):
                    nc.tensor.matmul(ps[:],
                                     ont_of[(qc3, jj)][:, mtl * P:(mtl + 1) * P],
                                     wps[jj][:, nt * QC:(nt + 1) * QC],
                                     start=(jj == 0), stop=(jj == 1))
                tb = xbt[:, ci, 0:QC]
                tail_sb[ci] = tb
                nc.vector.tensor_copy(tb, ps[:])

            tail_ps = {}

            def tail_jj2(ci, ps):
                qc3 = NQC - 1
                mtl, nt = ci // 2, ci % 2
                tail_ps[ci] = ps
                nc.tensor.matmul(ps,
                                 ont_of[(qc3, 2)][:, mtl * P:(mtl + 1) * P],
                                 wps[2][:, nt * QC:(nt + 1) * QC],
                                 start=True, stop=False)

            def tail_jj3(ci):
                qc3 = NQC - 1
                mtl, nt = ci // 2, ci % 2
                ps = tail_ps.pop(ci)
                nc.tensor.matmul(ps,
                                 ont_of[(qc3, 3)][:, mtl * P:(mtl + 1) * P],
                                 wps[3][:, nt * QC:(nt + 1) * QC],
                                 start=False, stop=True)
                ys = ysp.tile([P, QC], bf16, tag="ys", name="ys")
                nc.vector.tensor_add(ys[:], ps, tail_sb[ci])
                mt = (NQC - 1) * 4 + mtl
                nc.sync.dma_start(
                    out=y[mt * P:(mt + 1) * P, nt * QC:(nt + 1) * QC],
                    in_=ys[:])

            # ---- prefix: the chains feeding scores(0,0,*). The first K
            # chunk is split in column halves so exp0 fires before the Q
            # chain completes; emission order K-A, Q, K-B matches need order.
            psK = qk_proj(wk8t, 0, 0)
            psQ = qk_proj(wq8t, 0, 0)
            rope_chain(kt8, psK, 0, 0, c0=0, c1=256, pool_adds=2)
            rope_chain(qt8, psQ, 0, 0, pool_adds=2)
            rope_chain(kt8, psK, 0, 0, c0=256, c1=QC, pool_adds=2)
            # remaining K(m0) chunks; pool evac relieves qc0 DVE
            for qck in range(1, NQC):
                qk_rope(kt8, wk8t, 0, qck, pool_adds=3)

            # ---- steady state: flat (qc, j, kt) stream ----
            pend = deque()
            cur_ot = [None, None]

            def drain_one():
                j, qc, kt, a2t = pend.popleft()
                if kt == 0:
                    cur_ot[0] = otp.tile([P, 4, 65], f32, tag="otA", name="otA")
                    cur_ot[1] = otp.tile([P, 4, 65], f32, tag="otB", name="otB")
                av(j, kt, a2t, cur_ot[0], cur_ot[1])
                if kt == KT - 1:
                    normalize(j, qc, cur_ot[0], cur_ot[1])
                    cool[0] = 2

            # vproj placement: (qc, j) -> {kt: mt}
            VPROJ = {
                (0, 0): {3: 0, 5: 1, 7: 2, 9: 3, 11: 4, 13: 5, 14: 6, 15: 7},
                (0, 1): {0: 8, 2: 9, 4: 10, 6: 11, 8: 12, 10: 13},
                (0, 2): {0: 14, 2: 15},
            }
            # Q-chain prefetch: (qc, j) -> {kt: (m, for_qc)}
            QPREF = {}
            QPREF[(0, 3)] = {5: (0, 1)}
            QPREF[(1, 0)] = {1: (1, 1), 5: (2, 1)}
            QPREF[(1, 1)] = {3: (3, 1)}
            for q_ in range(2, NQC):
                QPREF[(q_, 0)] = {3: (2, q_)}
                QPREF[(q_, 1)] = {3: (3, q_)}
            for q_ in range(1, NQC - 1):
                QPREF.setdefault((q_, 2), {})[6] = (0, q_ + 1)
                QPREF[(q_, 2)][14] = (1, q_ + 1)

            cool = [0]

            def n_drains(qc, j, kt):
                if qc == 0:
                    if j == 0:
                        n = 0
                    elif j == 1:
                        n = 1 if kt >= 3 else 0
                    elif j == 2:
                        n = {0: 1, 1: 0, 2: 1, 3: 0}.get(kt, 2)
                    else:
                        n = 2 if len(pend) > 8 else 1
                    # a2 pool guard: force drains rather than stall scores
                    if len(pend) >= 23:
                        n = max(n, 2)
                    elif len(pend) >= 22:
                        n = max(n, 1)
                    return n
                if qc == NQC - 1 and j == 3:
                    return min(2, max(0, len(pend) - 4))
                # cooldown right after a group boundary hides the transpose
                # + evac2 latency on the freed accumulator banks
                if cool[0] > 0 and len(pend) < 20:
                    cool[0] -= 1
                    return 0
                return 2 if len(pend) > 5 else (1 if len(pend) > 4 else 0)

            def filler(qc, j, kt):
                vp = VPROJ.get((qc, j))
                if vp is not None and kt in vp:
                    vproj(vp[kt], wide=(j == 0))
                qp = QPREF.get((qc, j))
                if qp is not None and kt in qp:
                    m_, q_ = qp[kt]
                    qk_rope(qt8, wq8t, m_, q_)
                if qc == 0 and j == 0:
                    # K(m1)/Q0(m1) chains front-loaded so j1 scores can
                    # follow j0 seamlessly
                    if kt in (1, 3, 5, 7):
                        qk_rope(kt8, wk8t, 1, (kt - 1) // 2,
                                pool_adds=3)
                    elif kt == 9:
                        qk_rope(qt8, wq8t, 1, 0, pool_adds=3)
                if qc == 0 and j in (1, 2):
                    # K ropes for m=j+1
                    if kt in (1, 5, 9, 13):
                        qk_rope(kt8, wk8t, j + 1, (kt - 1) // 4,
                                pool_adds=3)
                    elif kt == 11:
                        qk_rope(qt8, wq8t, j + 1, 0,
                                pool_adds=3)
                elif qc >= 1 and j in (1, 2):
                    if kt % 2 == 1:
                        idx = (j - 1) * 8 + (kt - 1) // 2   # 0..15
                        mtl = idx // 4
                        nt = (idx // 2) % 2
                        outproj_half(qc - 1, mtl, nt, idx % 2)
                if qc == NQC - 1:
                    if j == 2 and kt in (8, 10, 12, 14):
                        tail_front((kt - 8) // 2)
                    elif j == 3 and kt in (8, 10, 12, 14):
                        tail_front(4 + (kt - 8) // 2)

            for qc in range(NQC):
                for j in range(4):
                    for kt in range(KT):
                        # drains + fillers first: their deps are satisfied, so
                        # the in-order PE works instead of blocking on the
                        # next scores psum bank
                        for _ in range(n_drains(qc, j, kt)):
                            if pend:
                                drain_one()
                        filler(qc, j, kt)
                        a2t = scores(j, qc, kt)
                        pend.append((j, qc, kt, a2t))
            while pend:
                drain_one()
            # post-stream: jj2 mms first (only need ont j2, overlap the last
            # normalize), then jj3 + fused add-evac per chunk. 6 psum lanes
            # (freed score banks + acc).
            lanes = []
            for i in range(2):
                sct = scp.tile([P, 2 * QC], f32, tag="sc", name=f"tl{i}")
                lanes.append(sct[:, 0:QC])
                lanes.append(sct[:, QC:2 * QC])
            for ci in range(6):
                tail_jj2(ci, lanes[ci] if ci < 4 else
                         accp.tile([P, QC], f32, tag="acc", name="tps2")[:])
            for ci in range(6):
                tail_jj3(ci)
            for ci in range(6, 8):
                tail_jj2(ci, lanes[ci - 6])
                tail_jj3(ci)

    nc.compile()
    return nc


_NC_CACHE = None


def _rope_tables():
    """cos/sin tables for the [evens(32)|odds(32)] per-head psum layout.
    Row p uses theta_(p%32); sin sign is - for the evens half."""
    thetas = 1000.0 ** (-2.0 * np.arange(1, 33, dtype=np.float64) / 64.0)
    pos = np.arange(1, T + 1, dtype=np.float64)
    args = pos[None, :] * thetas[:, None]          # [32, T]
    cos32 = np.cos(args)
    sin32 = np.sin(args)
    bf = ml_dtypes.bfloat16
    cos128 = np.tile(cos32, (4, 1)).astype(bf)
    sin128 = np.concatenate([-sin32, sin32, -sin32, sin32], axis=0).astype(bf)
    return np.ascontiguousarray(cos128), np.ascontiguousarray(sin128)


def kernel(x, W_attn, b_attn, W_proj, b_proj):
    global _NC_CACHE
    x = np.asarray(x, dtype=np.float32)
    W_attn = np.asarray(W_attn, dtype=np.float32)
    W_proj = np.asarray(W_proj, dtype=np.float32)
    b_proj = np.asarray(b_proj, dtype=np.float32)
    bf = ml_dtypes.bfloat16
    f8 = ml_dtypes.float8_e4m3
    cos128, sin128 = _rope_tables()

    in_maps = []
    for c in range(8):
        b = c // 2
        h0 = (c % 2) * 8
        qcols = np.concatenate([h * HD + _PERM for h in range(h0, h0 + 8)])
        vcols = np.arange(h0 * HD, (h0 + 8) * HD)

        xTb = np.ascontiguousarray(x[b].T)                       # [1024, 2048]
        # [P, kp, qc, slot, 512]
        xp8 = np.ascontiguousarray(
            xTb.reshape(4, 2, 128, 4, QC).transpose(2, 0, 3, 1, 4)
            .reshape(P, 4 * 4 * 2 * QC)
        ).astype(f8)
        xbig = np.ascontiguousarray(
            xTb.reshape(8, 128, T).transpose(1, 0, 2).reshape(P, 8 * T)
        ).astype(bf)

        def packw(Wsub):
            Ws = np.ascontiguousarray(Wsub[:, qcols]) * WPRE     # [1024, 512]
            return np.ascontiguousarray(
                Ws.reshape(4, 2, 128, 4, 128).transpose(2, 3, 0, 1, 4)
                .reshape(P, 4 * 4 * 2 * P)).astype(f8)

        Wvs = np.ascontiguousarray(W_attn[:, 2048:3072][:, vcols])  # [1024, 512]
        wvbig = np.ascontiguousarray(
            Wvs.reshape(8, 128, 512).transpose(1, 0, 2).reshape(P, 8 * 512)
        ).astype(bf)

        in_maps.append({
            "xp8": xp8,
            "xbig": xbig,
            "wq8": packw(W_attn[:, 0:1024]),
            "wk8": packw(W_attn[:, 1024:2048]),
            "wv": wvbig,
            "wp": np.ascontiguousarray(W_proj[vcols, :]).astype(bf),
            "csT": np.ascontiguousarray(np.concatenate([cos128, sin128],
                                                       axis=1)),
            "ident": np.eye(P, dtype=bf),
        })

    if _NC_CACHE is None:
        _NC_CACHE = _build_nc()
    import os
    trace = bool(os.environ.get("KERNEL_TRACE"))
    kw = {}
    if trace:
        tdir = os.environ.get("KERNEL_TRACE_DIR") or None
        kw = dict(trace=True, tmpdir=tdir)
    res = run_bass_kernel_spmd(_NC_CACHE, in_maps, list(range(8)), **kw)
    if trace and res.exec_time_ns is not None:
        print(f"HW exec time: {res.exec_time_ns} ns")
    out = np.empty((B, T, D), dtype=np.float32)
    for b in range(B):
        out[b] = (res.results[2 * b]["y"].astype(np.float32)
                  + res.results[2 * b + 1]["y"].astype(np.float32)
                  + b_proj[None, :])
    return out
